# revision 5
# baseline (speedup 1.0000x reference)
"""Multi-head causal attention (B=2, S=2048, D=1024, H=16, DK=DV=64) on 8 Trainium2
NeuronCores.

Sharding: 2-way batch x 4-way head-group. Core i handles batch i//4 and heads
[4*(i%4), 4*(i%4)+4). Each core projects q/k/v for its head group, runs causal
attention, and computes a partial output projection through its row-block of Wo.
The 4 partial outputs per batch are summed on the host (the all-reduce of the
row-sharded Wo output).

v3 design notes (driven by HW DVFS: PE clock ramps 0.65->2.4GHz only under
continuous execution; ANY gap resets the ramp and costs ~1us of recovery):
- V is projected directly into [k-pos, dv] layout (stationary = x tile,
  moving = weight block), eliminating the PE transposes + interleave copies
  of v2 entirely.
- The causal mask is applied by ACCUMULATING -1e9 into the diagonal PSUM
  block with one extra 128-row matmul (stationary=I, moving=NEG*tril) before
  the Exp; exp(-1e9)=0. This removes the Pool/DVE mask multiplies that sat on
  the ex->ov critical path in v2.
- Softmax denominators still come free from an all-ones column in vaug;
  reciprocal runs on DVE directly from the PSUM row; the broadcast rank-1
  matmul uses the fp32 reciprocal bitcast to f32r (1 cyc/row at N=512).
- Engine budget: ACT does ONLY Exp (~95us) + 2 oU copies/chunk; DVE does
  proj/oproj/oU drains + recip + oN muls (~70us); Pool does vaug copies +
  memsets (slack-tolerant); PE ~120us of real work is the critical path.
- Static wave schedule balances PE fillers against ACT load per chunk:
  all oproj units are deferred into the ACT-bound chunk 3; dummy transposes
  are sprinkled as always-ready clock-keepers where ACT would outrun PE.
"""
import sys

sys.path.insert(0, "/opt/trn_rl_repo")
import numpy as np

B, S, D = 2, 2048, 1024
H, DK, DV = 16, 64, 64
NCORES = 8
HG = 4          # head-group cores per batch
HPC = H // HG   # heads per core
HDC = HPC * DK  # 256 projection cols per core
P = 128         # partitions
CH = 512        # q-chunk size
VW = DV + 1     # v_aug width per head
NCH = S // CH   # 4 chunks
NST = S // P    # 16 s-tiles
ND = D // P     # 8 d-tiles
NM = HDC // P   # 2 head-pair tiles
NEG = -1e9

PRE_DUMMIES = 40        # PE warmup transposes before any real work
W0_DUMMIES = (5, 3, 3)  # dummies per dd-matmul for first V/K/Q proj groups
WAVE_DUMS = (2, 0, 1, 4)  # dummies per t-group in each wave (ACT slack)
TAIL_DUMS = 14          # dummies in the final drain


def build(nc, tile, mybir):
    BF16 = mybir.dt.bfloat16
    F32 = mybir.dt.float32
    F32R = mybir.dt.float32r
    Exp = mybir.ActivationFunctionType.Exp

    xvT = nc.dram_tensor("xvT", [D, S], BF16, kind="ExternalInput").ap()
    xkT = nc.dram_tensor("xkT", [D, S], BF16, kind="ExternalInput").ap()
    xqT = nc.dram_tensor("xqT", [D, S], BF16, kind="ExternalInput").ap()
    # wqkv columns ordered v | k | q
    wqkv = nc.dram_tensor("wqkv", [D, 3 * HDC], BF16, kind="ExternalInput").ap()
    wo = nc.dram_tensor("wo", [HDC, D], BF16, kind="ExternalInput").ap()
    maskM = nc.dram_tensor("maskM", [P, P], BF16, kind="ExternalInput").ap()
    identb = nc.dram_tensor("identb", [P, P], BF16, kind="ExternalInput").ap()
    onesb = nc.dram_tensor("onesb", [1, DK], BF16, kind="ExternalInput").ap()
    ident = nc.dram_tensor("ident", [P, P], F32R, kind="ExternalInput").ap()
    out = nc.dram_tensor("out", [S, D], BF16, kind="ExternalOutput").ap()

    with tile.TileContext(nc) as tc:
        from contextlib import ExitStack
        with ExitStack() as ctx:
            wp = ctx.enter_context(tc.tile_pool(name="wp", bufs=1))
            exp_ = ctx.enter_context(tc.tile_pool(name="exp", bufs=12))
            sp = ctx.enter_context(tc.tile_pool(name="sp", bufs=2))
            obp = ctx.enter_context(tc.tile_pool(name="obp", bufs=3))
            sc_ps = ctx.enter_context(tc.tile_pool(name="sc_ps", bufs=3, space="PSUM"))
            ov_ps = ctx.enter_context(tc.tile_pool(name="ov_ps", bufs=4, space="PSUM"))
            aux_ps = ctx.enter_context(tc.tile_pool(name="aux_ps", bufs=1, space="PSUM"))

            # ---- persistent SBUF tiles ----
            idt = wp.tile([P, P], F32R, name="idt")
            mM = wp.tile([P, P], BF16, name="mM")
            idb = wp.tile([P, P], BF16, name="idb")
            onb = wp.tile([1, DK], BF16, name="onb")
            wqkv_t = [wp.tile([P, 3 * HDC], BF16, name=f"wqkv{i}") for i in range(ND)]
            wo_t = [wp.tile([P, D], BF16, name=f"wo{i}") for i in range(NM)]
            xts = {tn: [[wp.tile([P, 2 * CH], BF16, name=f"x{tn}_{hf}_{dd}")
                         for dd in range(ND)] for hf in range(2)]
                   for tn in ("v", "k", "q")}
            qT = [wp.tile([P, S], BF16, name=f"qT{m}") for m in range(NM)]
            kT = [wp.tile([P, S], BF16, name=f"kT{m}") for m in range(NM)]
            oU = [wp.tile([P, S], BF16, name=f"oU{m}") for m in range(NM)]
            oN = [wp.tile([P, S], BF16, name=f"oN{m}") for m in range(NM)]
            vaug = [wp.tile([P, HPC * VW], BF16, name=f"vaug{t}") for t in range(NST)]
            scr = wp.tile([1, 16], BF16, name="scr")

            # ---- DMA issue order (SP queue = strict FIFO priority) ----
            nc.sync.dma_start(idt[:], ident[:, :])
            nc.sync.dma_start(idb[:], identb[:, :])
            nc.sync.dma_start(mM[:], maskM[:, :])
            nc.sync.dma_start(onb[:], onesb[:, :])
            xsrc = {"v": xvT, "k": xkT, "q": xqT}
            WSEL = {"v": 0, "k": HDC, "q": 2 * HDC}

            def dma_w(tn):
                lo = WSEL[tn]
                for dd in range(ND):
                    nc.sync.dma_start(wqkv_t[dd][:, lo:lo + HDC],
                                      wqkv[dd * P:(dd + 1) * P, lo:lo + HDC])

            def dma_x(tn, hf, dd):
                nc.sync.dma_start(xts[tn][hf][dd][:],
                                  xsrc[tn][dd * P:(dd + 1) * P,
                                           hf * 2 * CH:(hf + 1) * 2 * CH])

            dma_w("v")
            for dd in range(ND):
                dma_x("v", 0, dd)
            dma_w("k")
            for dd in range(ND):
                dma_x("k", 0, dd)
            dma_w("q")
            for dd in range(ND):
                dma_x("q", 0, dd)
            for i in range(NM):
                nc.sync.dma_start(wo_t[i][:], wo[i * P:(i + 1) * P, :])
            for tn in ("v", "k", "q"):
                for dd in range(ND):
                    dma_x(tn, 1, dd)

            # ACT: preload the Exp table during the DMA window
            nc.scalar.activation(scr[:], idb[0:1, 0:16], Exp)
            # vaug all-ones denominator columns (Pool memset)
            for t in range(NST):
                nc.gpsimd.memset(vaug[t][:, DV::VW], 1.0)

            # ---- emission helpers ----
            def dummy():
                dum = sc_ps.tile([P, P], F32R, name="dum", tag="sc")
                nc.tensor.transpose(dum[:], idt[:], idt[:])

            def vproj(st, dums=0):
                """project V for k-tile st directly into [kp, dv] -> vaug."""
                hf, sub = st // 8, (st % 8) * P
                pp = aux_ps.tile([P, HDC], F32, name="vp", tag="aux")
                for dd in range(ND):
                    nc.tensor.matmul(pp[:], xts["v"][hf][dd][:, sub:sub + P],
                                     wqkv_t[dd][:, 0:HDC],
                                     start=(dd == 0), stop=(dd == ND - 1))
                    for _ in range(dums):
                        dummy()
                dst = vaug[st][:].rearrange("p (h x) -> p h x", x=VW)[:, :, 0:DV]
                src = pp[:].rearrange("p (h x) -> p h x", x=DV)
                nc.vector.tensor_copy(dst, src)

            def qkproj(tn, m, c, dums=0, on_act=False):
                """project x_tn chunk c through W block m -> qT/kT cols."""
                pp = aux_ps.tile([P, CH], F32, name="pg", tag="aux")
                hf, sub = c // 2, (c % 2) * CH
                w0 = WSEL[tn] + m * P
                for dd in range(ND):
                    nc.tensor.matmul(pp[:], wqkv_t[dd][:, w0:w0 + P],
                                     xts[tn][hf][dd][:, sub:sub + CH],
                                     start=(dd == 0), stop=(dd == ND - 1))
                    for _ in range(dums):
                        dummy()
                dst = (qT if tn == "q" else kT)[m][:, c * CH:(c + 1) * CH]
                if on_act:
                    nc.scalar.copy(dst, pp[:])
                else:
                    nc.vector.tensor_copy(dst, pp[:])

            def sc_unit(h, c, t):
                """scores tile t for head h, chunk c -> exp'd ex tile."""
                mi, ri = h // 2, (h % 2) * DK
                r = t - 4 * c
                lo = max(r, 0) * P
                scp = sc_ps.tile([P, CH], F32, name="scp", tag="sc")
                nc.tensor.matmul(
                    scp[:, lo:CH],
                    kT[mi][ri:ri + DK, t * P:(t + 1) * P],
                    qT[mi][ri:ri + DK, c * CH + lo:(c + 1) * CH],
                    start=True, stop=(r < 0))
                if r >= 0:
                    nc.tensor.matmul(scp[:, lo:lo + P], idb[:], mM[:],
                                     start=False, stop=True)
                ex = exp_.tile([P, CH], BF16, name="ex", tag="ex")
                nc.scalar.activation(ex[:, lo:CH], scp[:, lo:CH], Exp)
                return ex, lo

            def ov_unit(ovt, h, c, t, ex, lo, nt):
                nc.tensor.matmul(ovt[:, lo:CH], vaug[t][:, h * VW:(h + 1) * VW],
                                 ex[:, lo:CH], start=(t == 0), stop=(t == nt - 1))

            recfs = {}

            def tail_drain(ovt, c, h):
                """recip of denominator row; stash unnormalized numerator."""
                dn = sp.tile([1, CH], F32, name="dn", tag="dn", bufs=3)
                nc.vector.tensor_copy(dn[:], ovt[DV:DV + 1, :])
                recf = sp.tile([1, CH], F32, name="recf", tag="recf", bufs=3)
                nc.vector.reciprocal_approx_fast(out=recf[:], in_=dn[:])
                recb = sp.tile([1, CH], BF16, name="recb", tag="recb", bufs=8)
                nc.vector.tensor_copy(recb[:], recf[:])
                recfs[(c, h)] = recb
                mi, ri = h // 2, (h % 2) * DK
                dst = oU[mi][ri:ri + DK, c * CH:(c + 1) * CH]
                if h >= 2:
                    nc.scalar.copy(dst, ovt[0:DV, :])
                else:
                    nc.vector.tensor_copy(dst, ovt[0:DV, :])

            def rank1mult(c, h):
                mi, ri = h // 2, (h % 2) * DK
                rb = aux_ps.tile([DK, CH], F32, name="rb", tag="aux")
                nc.tensor.matmul(rb[:], onb[:], recfs[(c, h)][:],
                                 start=True, stop=True)
                sl = slice(c * CH, (c + 1) * CH)
                nc.vector.tensor_mul(oN[mi][ri:ri + DK, sl],
                                     oU[mi][ri:ri + DK, sl], rb[:])

            def oproj(c, j):
                st = 4 * c + j
                ob = obp.tile([P, D], BF16, name="ob", tag="ob")
                for n2 in range(D // CH):
                    pp = sc_ps.tile([P, CH], F32, name="pout", tag="sc")
                    for m in range(NM):
                        nc.tensor.matmul(pp[:], oN[m][:, st * P:(st + 1) * P],
                                         wo_t[m][:, n2 * CH:(n2 + 1) * CH],
                                         start=(m == 0), stop=(m == NM - 1))
                    nc.vector.tensor_copy(ob[:, n2 * CH:(n2 + 1) * CH], pp[:])
                nc.sync.dma_start(out[st * P:(st + 1) * P, :], ob[:])

            # ---- W0: warmup + chunk-0 producers ----
            for _ in range(PRE_DUMMIES):
                dummy()
            for st in range(4):
                vproj(st, dums=W0_DUMMIES[0] if st < 2 else 1)
            for m in range(NM):
                qkproj("k", m, 0, dums=W0_DUMMIES[1] if m == 0 else 1)
            for m in range(NM):
                qkproj("q", m, 0, dums=W0_DUMMIES[2] if m == 0 else 1)

            # ---- waves: chain(c) with interleaved fillers ----
            # oproj(c) is valid from wave c+1 on; defer per OPROJ_AT schedule.
            OPROJ_AT = {1: [(0, 0)],
                        2: [(0, 1), (0, 2), (1, 0), (1, 1)],
                        3: [(0, 3), (1, 2), (1, 3), (2, 0), (2, 1), (2, 2)]}
            # remaining: (2,3) + all of chunk 3 go to the final drain

            def wave_units(c):
                units = []
                if c + 1 < NCH:
                    for st in range(4 * (c + 1), 4 * (c + 1) + 4):
                        units.append(lambda st=st: vproj(st))
                if c >= 1:
                    for h in range(HPC):
                        units.append(lambda h=h: rank1mult(c - 1, h))
                if c + 1 < NCH:
                    for tn in ("k", "q"):
                        for m in range(NM):
                            units.append(
                                lambda tn=tn, m=m: qkproj(tn, m, c + 1))
                for (cc, j) in OPROJ_AT.get(c, []):
                    units.append(lambda cc=cc, j=j: oproj(cc, j))
                return units

            for c in range(NCH):
                nt = 4 * c + 4
                last = c == NCH - 1
                units = wave_units(c)
                done = 0
                ovts = [ov_ps.tile([DV + 1, CH], F32, name=f"ov{h}", tag="ov")
                        for h in range(HPC)]
                exq = []  # pending (t, [per-head (ex, lo)]) with lag 2
                for t in range(nt):
                    cur = [sc_unit(0, c, t), sc_unit(1, c, t)]
                    if len(exq) >= 2:
                        ot, oexs = exq.pop(0)
                        for h in range(HPC):
                            ov_unit(ovts[h], h, c, ot, *oexs[h], nt)
                    cur += [sc_unit(2, c, t), sc_unit(3, c, t)]
                    exq.append((t, cur))
                    for _ in range(WAVE_DUMS[c]):
                        dummy()
                    want = len(units) * (t + 1) // nt
                    while done < want:
                        units[done]()
                        done += 1
                for ot, oexs in exq:
                    for h in range(HPC):
                        ov_unit(ovts[h], h, c, ot, *oexs[h], nt)
                while done < len(units):
                    units[done]()
                    done += 1
                for h in range(HPC):
                    tail_drain(ovts[h], c, h)

            # ---- final drain ----
            c = NCH - 1
            oproj(2, 3)
            for h in range(HPC):
                rank1mult(c, h)
                for _ in range(2):
                    dummy()
            for _ in range(TAIL_DUMS):
                dummy()
            for j in range(4):
                oproj(c, j)
    nc.compile()
    return nc


_NC_CACHE = {}
LAST_RESULT = None


def _get_nc():
    if "nc" not in _NC_CACHE:
        import concourse.tile as tile
        import concourse.mybir as mybir
        from concourse import bacc
        nc = bacc.Bacc("TRN2", target_bir_lowering=False, num_devices=NCORES)
        _NC_CACHE["nc"] = build(nc, tile, mybir)
    return _NC_CACHE["nc"]


def kernel(Q, K, V, Wq, Wk, Wv, Wo):
    import ml_dtypes
    from concourse.bass_utils import run_bass_kernel_spmd
    BF = ml_dtypes.bfloat16

    Q = np.asarray(Q, dtype=np.float32)
    K = np.asarray(K, dtype=np.float32)
    V = np.asarray(V, dtype=np.float32)
    Wq = np.asarray(Wq, dtype=np.float32) * np.float32(1.0 / np.sqrt(DK))
    Wk = np.asarray(Wk, dtype=np.float32)
    Wv = np.asarray(Wv, dtype=np.float32)
    Wo = np.asarray(Wo, dtype=np.float32)

    QT = [np.ascontiguousarray(Q[b].T).astype(BF) for b in range(B)]
    KT = [np.ascontiguousarray(K[b].T).astype(BF) for b in range(B)]
    VT = [np.ascontiguousarray(V[b].T).astype(BF) for b in range(B)]

    i = np.arange(P)[:, None]
    j = np.arange(P)[None, :]
    maskM = np.where(i > j, np.float32(NEG), np.float32(0)).astype(BF)
    identb = np.eye(P, dtype=np.float32).astype(BF)
    onesb = np.ones((1, DK), dtype=BF)
    ident = np.eye(P, dtype=np.float32)

    in_maps = []
    for core in range(NCORES):
        b, g = core // HG, core % HG
        cs = slice(g * HDC, (g + 1) * HDC)
        in_maps.append({
            "xqT": QT[b], "xkT": KT[b], "xvT": VT[b],
            "wqkv": np.ascontiguousarray(
                np.concatenate([Wv[:, cs], Wk[:, cs], Wq[:, cs]],
                               axis=1)).astype(BF),
            "wo": np.ascontiguousarray(Wo[cs, :]).astype(BF),
            "maskM": maskM, "identb": identb, "onesb": onesb, "ident": ident,
        })

    nc = _get_nc()
    res = run_bass_kernel_spmd(nc, in_maps, core_ids=list(range(NCORES)))
    global LAST_RESULT
    LAST_RESULT = res

    acc = np.zeros((B, S, D), dtype=np.float64)
    for core in range(NCORES):
        acc[core // HG] += res.results[core]["out"].astype(np.float64)
    return acc.astype(np.float32)
